# revision 9
# baseline (speedup 1.0000x reference)
"""Multi-head self-attention (causal) Trainium2 Bass/Tile kernel, 8-way SPMD.

Sharding: data-parallel over batch (4) x tensor-parallel over heads (2 groups
of 8 heads).  Core c handles batch c//2, head-group c%2.  Each core computes
q/k/v projections for its 512 local features, causal attention for its 8
heads, and a partial o-projection (contraction over its 512 features of the
attention output) giving a full-shape [S, D] partial (bf16) that the host
sums per batch pair.

All matmul operands are bf16 (fp32 PSUM accumulation); softmax runs without
max-subtraction (scores ~ N(0,1) after the 1/8 scale, no overflow risk), with
exp on the scalar engine and the row-sum folded into the AV matmul via a ones
column appended to V.  Host pre-transposes inputs so no on-chip transposes
are needed:
  qT[e,s]  = wqT.T @ xT        (lhsT=wqT[d,e], rhs=xT[d,s])
  scoresT[sk,sq] = kT.T @ qT   (lhsT=kT[dk,sk], rhs=qT[dk,sq], K=64)
  avT[dk+1,sq]   = vaug.T @ expT  (lhsT=vaug[sk,65], rhs=expT[sk,sq])
  y[s,e]   = outT.T @ woT      (lhsT=outT[d,s], rhs=woT[d,e])

The attention inner loop is ACT(exp)-cadence-limited (~1.1us per key tile vs
~0.8us of PE fill), so all projection / o-projection matmuls are DRIPPED one
instruction at a time between attention matmuls by a tiny work-queue
scheduler; slot boundaries force-drain whatever an upcoming slot depends on.

Causal structure: off-diagonal key tiles are computed full-width; the 4
diagonal-strip tiles of each query group only compute queries >= the key
tile's start (narrowed scores / exp / AV), with a single shared [128,128]
upper-triangular mask applied to the one partially-masked 128-query chunk.
PSUM per-element has_written semantics make the narrowed AV accumulation
correct in any order (first matmul start=True pending-zeroes the bank).
"""

from contextlib import ExitStack

import numpy as np
import ml_dtypes

import concourse.bass as bass
import concourse.tile as tile
from concourse import bacc, mybir
from concourse._compat import with_exitstack
from concourse.bass_utils import run_bass_kernel_spmd

B, S, D, H = 4, 2048, 1024, 16
DK = D // H          # 64
E = 512              # local features per core (8 heads)
HL = 8               # local heads
NCORES = 8
NDT = D // 128       # 8 d-tiles
NET = E // 128       # 4 e-tiles
NST = S // 128       # 16 s-tiles
NQG = S // 512       # 4 query groups

LAG = 2              # exp -> AV pipeline lag (in key tiles)
N_WARM = 8           # HAM warm-up matmuls
DRIP_RATE = 1.2      # drip steps per off-diagonal key tile

F32 = mybir.dt.float32
BF16 = mybir.dt.bfloat16
bf16 = ml_dtypes.bfloat16

_compiled = None
last_results = None  # test harness introspection


class Drip:
    """FIFO work queue of single-matmul steps, dripped between attention
    matmuls at a budgeted rate; need() force-drains through a keyed item."""

    def __init__(self):
        self.q = []          # list of [key, steps]
        self.done = set()
        self.budget = 0.0

    def add(self, key, steps):
        self.q.append([key, list(steps)])

    def _run_one(self):
        while self.q and not self.q[0][1]:
            self.done.add(self.q.pop(0)[0])
        if not self.q:
            return False
        self.q[0][1].pop(0)()
        if not self.q[0][1]:
            self.done.add(self.q.pop(0)[0])
        return True

    def need(self, key):
        if key in self.done:
            return
        keys = [k for k, _ in self.q]
        assert key in keys, f"drip: {key} not queued"
        while key not in self.done:
            assert self._run_one()

    def drip(self, rate):
        self.budget += rate
        while self.budget >= 1.0:
            if not self._run_one():
                self.budget = 0.0
                return
            self.budget -= 1.0

    def drain_all(self):
        while self._run_one():
            pass


@with_exitstack
def _mhsa_kernel(ctx: ExitStack, tc: tile.TileContext, y, xT, wqT, wkT, wvT,
                 woT, tri):
    nc = tc.nc

    consts = ctx.enter_context(tc.tile_pool(name="consts", bufs=1))
    ex_pool = ctx.enter_context(tc.tile_pool(name="ex", bufs=10))
    rec_pool = ctx.enter_context(tc.tile_pool(name="rec", bufs=2))
    y_pool = ctx.enter_context(tc.tile_pool(name="ysb", bufs=3))
    sc_pool = ctx.enter_context(tc.tile_pool(name="sc", bufs=2, space="PSUM"))
    pj_pool = ctx.enter_context(tc.tile_pool(name="pj", bufs=2, space="PSUM"))
    av_pool = ctx.enter_context(tc.tile_pool(name="av", bufs=2, space="PSUM"))

    def ctile(shape, dt_, tg):
        return consts.tile(shape, dt_, tag=tg, name=tg)

    # ---- persistent SBUF tiles -------------------------------------------
    xT_t = ctile([128, NDT * S], BF16, "xTt")          # [p, dt*2048 + s]
    wqT_t = ctile([128, NDT * E], BF16, "wqTt")        # [p, dt*512 + e]
    wkT_t = ctile([128, NDT * E], BF16, "wkTt")
    wvT_t = ctile([128, NDT * E], BF16, "wvTt")
    woT_t = ctile([128, NET * D], BF16, "woTt")        # [p, et*1024 + d]
    qT_t = [ctile([128, S], BF16, f"qT{i}") for i in range(NET)]
    kT_t = [ctile([128, S], BF16, f"kT{i}") for i in range(NET)]
    vaug_t = [ctile([128, HL * (DK + 1)], BF16, f"vaug{i}") for i in range(NST)]
    outT_t = [ctile([128, S], BF16, f"outT{i}") for i in range(NET)]
    tri_t = ctile([128, 128], BF16, "tri")
    warm = ctile([128, 512], BF16, "warm")

    def x_sl(dt_, a, b):
        return xT_t[:, dt_ * S + a:dt_ * S + b]

    def wq_sl(dt_, et):
        return wqT_t[:, dt_ * E + et * 128:dt_ * E + (et + 1) * 128]

    def wk_sl(dt_, et):
        return wkT_t[:, dt_ * E + et * 128:dt_ * E + (et + 1) * 128]

    def wv_sl(dt_):
        return wvT_t[:, dt_ * E:(dt_ + 1) * E]

    def wo_sl(et, hf):
        return woT_t[:, et * D + hf * 512:et * D + (hf + 1) * 512]

    # ---- input loads: merged DMAs spread over the 3 DMA-capable queues ---
    # gpsimd: xT cols [0:512] per d-tile (feeds the first v chains asap)
    # sync: wvT (first compute dependency), then xT col merges
    # scalar: wqT, wkT, tri, woT (scalar queue is free until the first exp)
    # warm tile memset on the (idle) vector queue so the HAM warm-up
    # matmuls are not stuck behind DMA issues on gpsimd
    nc.vector.memset(warm, 0.0)

    xT3d = xT_t.rearrange("p (t s) -> p t s", s=S)
    xTsrc = xT.rearrange("(t p) s -> p t s", p=128)
    nc.sync.dma_start(
        out=wvT_t.rearrange("p (t e) -> p t e", e=E),
        in_=wvT.rearrange("(t p) e -> p t e", p=128))
    for i in range(NDT):
        nc.gpsimd.dma_start(out=x_sl(i, 0, 512), in_=xT[i * 128:(i + 1) * 128, 0:512])
    nc.scalar.dma_start(
        out=wqT_t.rearrange("p (t e) -> p t e", e=E),
        in_=wqT.rearrange("(t p) e -> p t e", p=128))
    nc.scalar.dma_start(
        out=wkT_t.rearrange("p (t e) -> p t e", e=E),
        in_=wkT.rearrange("(t p) e -> p t e", p=128))
    nc.sync.dma_start(out=xT3d[:, :, 512:1024], in_=xTsrc[:, :, 512:1024])
    nc.sync.dma_start(out=xT3d[:, :, 1024:2048], in_=xTsrc[:, :, 1024:2048])
    nc.scalar.dma_start(out=tri_t, in_=tri)
    nc.scalar.dma_start(
        out=woT_t.rearrange("p (t d) -> p t d", d=D),
        in_=woT.rearrange("(t p) d -> p t d", p=128))

    # ---- PE warm-up: HAM releases after ~3.4us of sustained matmuls ------
    for _ in range(N_WARM):
        wps = pj_pool.tile([128, 512], F32, tag="pj", name="wps")
        nc.tensor.matmul(wps, lhsT=warm[:, 0:128], rhs=warm,
                         start=True, stop=True)

    # ---- projection chains (single-matmul drip steps) --------------------
    def qk_steps(wsl, dst, et, scg, hf):
        steps = []
        state = {}
        for dt_ in range(NDT):
            def step(dt_=dt_):
                if dt_ == 0:
                    state[0] = pj_pool.tile([128, 512], F32, tag="pj",
                                            name="pj")
                ps = state[0]
                s0 = scg * 1024 + hf * 512
                nc.tensor.matmul(
                    ps, lhsT=wsl(dt_, et), rhs=x_sl(dt_, s0, s0 + 512),
                    start=(dt_ == 0), stop=(dt_ == NDT - 1))
                if dt_ == NDT - 1:
                    nc.vector.tensor_copy(dst[et][:, s0:s0 + 512], ps)
            steps.append(step)
        return steps

    def v_steps(st):
        steps = []
        state = {}
        for dt_ in range(NDT):
            def step(dt_=dt_):
                if dt_ == 0:
                    state[0] = pj_pool.tile([128, 512], F32, tag="pj",
                                            name="pj")
                ps = state[0]
                nc.tensor.matmul(
                    ps, lhsT=x_sl(dt_, st * 128, (st + 1) * 128), rhs=wv_sl(dt_),
                    start=(dt_ == 0), stop=(dt_ == NDT - 1))
                if dt_ == NDT - 1:
                    nc.vector.memset(vaug_t[st], 1.0)
                    nc.vector.tensor_copy(
                        vaug_t[st].rearrange("p (h c) -> p h c", c=65)[:, :, 0:64],
                        ps.rearrange("p (h c) -> p h c", c=64))
            steps.append(step)
        return steps

    def o_steps(st):
        steps = []
        state = {}
        for hf in range(2):
            for et in range(NET):
                def step(hf=hf, et=et, st=st):
                    if et == 0:
                        state[hf] = pj_pool.tile([128, 512], F32, tag="pj",
                                                 name="pj")
                    ps = state[hf]
                    nc.tensor.matmul(
                        ps, lhsT=outT_t[et][:, st * 128:(st + 1) * 128],
                        rhs=wo_sl(et, hf),
                        start=(et == 0), stop=(et == NET - 1))
                    if et == NET - 1:
                        ysb = y_pool.tile([128, 512], BF16, tag="ysb",
                                          name="ysb")
                        nc.vector.tensor_copy(ysb, ps)
                        eng = nc.gpsimd if (st + hf) % 2 == 0 else nc.sync
                        eng.dma_start(
                            out=y[st * 128:(st + 1) * 128,
                                  hf * 512:(hf + 1) * 512],
                            in_=ysb)
                steps.append(step)
        return steps

    # Softmax denominators bounce through DRAM: DVE can only write at
    # 32-aligned base partitions, and SBUF APs cannot have a step-0
    # partition dim (needed for the broadcast) — DRAM APs can do both.
    sums_dram = nc.dram_tensor("sums_bounce", [NQG, HL, 512], F32).ap()
    rec_dram = nc.dram_tensor("rec_bounce", [NQG, HL, 512], BF16).ap()

    # ones2: selector for the final pair's reciprocal broadcast matmul —
    # bc[j, :] = recb2[0, :] for j<64 (head A) and recb2[32, :] for j>=64
    # (head B).  Rows 0/32 because the DVE can only write at 32-aligned
    # partitions; K padded to 64 (a K=33 matmul wedges the exec unit).
    ones2 = ctile([64, 128], BF16, "ones2")
    nc.vector.memset(ones2, 0.0)
    nc.vector.memset(ones2[0:1, 0:64], 1.0)
    nc.vector.memset(ones2[32:33, 64:128], 1.0)

    sched = Drip()

    # ---- attention: one global pipeline over all (qg, hp, kt) -------------
    # Heads hA=2*hp (partitions 0:64) and hB=2*hp+1 (64:128) share each
    # score tile: [:, 0:512]=A, [:, 512:1024]=B for one key tile kt.  The
    # K=64 score matmuls for A and B land on disjoint PE row groups (base
    # partition 0 vs 64) and run concurrently.  outT stays UNNORMALIZED;
    # denominators are collected and normalization is batched per qg.
    # Diagonal-strip tiles (kt-4*qg = j >= 0) are narrowed to queries
    # >= 128*j within the group.  The exp->AV lag spans slot boundaries so
    # the PE never waits on a fresh exp at a slot transition; a slot's
    # stash (and its boundary actions) runs when its last AV pops, ~LAG
    # key tiles into the next slot.
    pend = []

    def emit_av(it):
        kt, ex, qo, avA, avB, qg, hp = it
        nk = 4 * qg + 4
        for av, h in ((avA, 2 * hp), (avB, 2 * hp + 1)):
            nc.tensor.matmul(
                av[:, qo:512],
                lhsT=vaug_t[kt][:, h * 65:h * 65 + 65],
                rhs=ex[:, (h & 1) * 512 + qo:((h & 1) + 1) * 512],
                start=(kt == 0), stop=(kt == nk - 1),
            )
        if kt == nk - 1:
            _stash(hp, qg, hp, avA, avB)
            boundary(qg, hp)

    def attn(hp, qg):
        ti = hp
        nk = 4 * qg + 4
        avA = av_pool.tile([65, 512], F32, tag="av", name="avA")
        avB = av_pool.tile([65, 512], F32, tag="av", name="avB")
        for kt in range(nk):
            j = kt - 4 * qg          # >=0 on the diagonal strip
            qo = 128 * j if j > 0 else 0
            ps = sc_pool.tile([128, 1024], F32, tag="sc", name="ps")
            for po in (0, 64):
                nc.tensor.matmul(
                    ps[:, (po // 64) * 512 + qo:(po // 64 + 1) * 512],
                    lhsT=kT_t[ti][po:po + 64, kt * 128:(kt + 1) * 128],
                    rhs=qT_t[ti][po:po + 64, qg * 512 + qo:(qg + 1) * 512],
                    start=True, stop=True,
                )
            ex = ex_pool.tile([128, 1024], BF16, tag="ex", name="ex")
            if qo == 0:
                nc.scalar.activation(out=ex, in_=ps,
                                     func=mybir.ActivationFunctionType.Exp,
                                     scale=0.125)
            else:
                for half in range(2):
                    a = half * 512 + qo
                    b = (half + 1) * 512
                    nc.scalar.activation(
                        out=ex[:, a:b], in_=ps[:, a:b],
                        func=mybir.ActivationFunctionType.Exp, scale=0.125)
            if j >= 0:  # triangular mask on the one partial 128-q chunk
                for half in range(2):
                    a = half * 512 + qo
                    nc.vector.tensor_mul(ex[:, a:a + 128], ex[:, a:a + 128],
                                         tri_t)
            pend.append((kt, ex, qo, avA, avB, qg, hp))
            rate = DRIP_RATE if j < 1 else max(0.2, DRIP_RATE - 0.35 * j)
            sched.drip(rate)
            while len(pend) > LAG:
                emit_av(pend.pop(0))

    def _stash(hp, qg, ti, avA, avB):
        # stash unnormalized outputs + denominators; release av quickly
        hA, hB = 2 * hp, 2 * hp + 1
        if qg == NQG - 1 and hp == HL // 2 - 1:
            # final pair: no attention left to hide the DRAM-bounce latency
            # behind, so normalize inline via reciprocal + PE broadcast
            stg2 = rec_pool.tile([64, 512], F32, tag="stg2", name="stg2")
            nc.vector.memset(stg2, 1.0)
            for av, po, row in ((avA, 0, 0), (avB, 64, 32)):
                nc.vector.tensor_copy(
                    outT_t[ti][po:po + 64, qg * 512:(qg + 1) * 512],
                    av[0:64, :])
                nc.vector.tensor_copy(stg2[row:row + 1, :], av[64:65, :])
            rec2 = rec_pool.tile([64, 512], F32, tag="rec2", name="rec2")
            nc.vector.reciprocal_approx_fast(out=rec2, in_=stg2)
            recb2 = rec_pool.tile([64, 512], BF16, tag="recb2", name="recb2")
            nc.vector.tensor_copy(recb2, rec2)
            bc = av_pool.tile([128, 512], F32, tag="av", name="bc")
            nc.tensor.matmul(bc, lhsT=ones2, rhs=recb2, start=True, stop=True)
            for po in (0, 64):
                sl = outT_t[ti][po:po + 64, qg * 512:(qg + 1) * 512]
                nc.vector.tensor_mul(sl, sl, bc[po:po + 64, :])
        else:
            for av, h, po in ((avA, hA, 0), (avB, hB, 64)):
                nc.vector.tensor_copy(
                    outT_t[ti][po:po + 64, qg * 512:(qg + 1) * 512],
                    av[0:64, :])
                stg = rec_pool.tile([1, 512], F32, tag="stg", name="stg",
                                    bufs=4)
                nc.vector.tensor_copy(stg, av[64:65, :])
                nc.sync.dma_start(out=sums_dram[qg, h], in_=stg)

    # ---- batched normalization (DRAM-bounce broadcast) -------------------
    def _norm_heads(qg, heads):
        h0, nh = heads[0], len(heads)
        sums = rec_pool.tile([nh, 512], F32, tag=f"sums{nh}", name="sums")
        nc.sync.dma_start(out=sums, in_=sums_dram[qg, h0:h0 + nh])
        rec = rec_pool.tile([nh, 512], F32, tag=f"rec{nh}", name="rec")
        nc.vector.reciprocal_approx_fast(out=rec, in_=sums)
        recb = rec_pool.tile([nh, 512], BF16, tag=f"recb{nh}", name="recb")
        nc.vector.tensor_copy(recb, rec)
        nc.sync.dma_start(out=rec_dram[qg, h0:h0 + nh], in_=recb)
        for h in heads:
            ti, po = h // 2, 64 * (h % 2)
            # walrus requires SBUF tensor_tensor inputs to share the start
            # partition, so land the broadcast at the same partition range
            bcs = rec_pool.tile([128, 512], BF16, tag="bcs", name="bcs")
            nc.sync.dma_start(
                out=bcs[po:po + 64, :],
                in_=rec_dram[qg, h:h + 1, :].to_broadcast([64, 512]))
            sl = outT_t[ti][po:po + 64, qg * 512:(qg + 1) * 512]
            nc.vector.tensor_mul(sl, sl, bcs[po:po + 64, :])

    def normalize(qg):
        _norm_heads(qg, list(range(HL)))

    def normalize_pair(qg, hp):
        _norm_heads(qg, [2 * hp, 2 * hp + 1])

    # post-slot boundary actions, keyed by (qg, hp) whose stash just ran
    def boundary(qg, hp):
        if qg == 1 and hp == 0:
            normalize(0)
            for st in (0, 1, 2, 3):
                sched.add(("o", st), o_steps(st))
        elif qg == 2 and hp == 0:
            normalize(1)
            for st in (4, 5, 6, 7):
                sched.add(("o", st), o_steps(st))
        elif qg == 3 and hp == 0:
            normalize(2)
            for st in (8, 9, 10, 11):
                sched.add(("o", st), o_steps(st))
        if qg == 3 and hp < 3:
            normalize_pair(3, hp)

    # ---- program order ----------------------------------------------------
    # Pre-loop: everything slot (0,0) depends on.  The drip queue holds the
    # remaining projections in the order later slots need them; oproj steps
    # are appended once their query group is normalized.
    def addq(et, scg, hf):
        sched.add(("q", et, scg, hf), qk_steps(wq_sl, qT_t, et, scg, hf))

    def addk(et, scg, hf):
        sched.add(("k", et, scg, hf), qk_steps(wk_sl, kT_t, et, scg, hf))

    for st in range(4):
        sched.add(("v", st), v_steps(st))
    addq(0, 0, 0)
    addk(0, 0, 0)
    sched.need(("v", 3))
    sched.need(("q", 0, 0, 0))
    sched.need(("k", 0, 0, 0))

    for et in (1, 2, 3):
        addq(et, 0, 0)
        addk(et, 0, 0)
    for st in (4, 5, 6, 7):
        sched.add(("v", st), v_steps(st))
    for et in (0, 1, 2, 3):
        addk(et, 0, 1)
        addq(et, 0, 1)
    for st in (8, 9, 10, 11):
        sched.add(("v", st), v_steps(st))
    for et in (0, 1, 2, 3):
        addq(et, 1, 0)
        addk(et, 1, 0)
    for st in (12, 13, 14, 15):
        sched.add(("v", st), v_steps(st))
    for et in (0, 1, 2, 3):
        addk(et, 1, 1)
        addq(et, 1, 1)

    for qg in range(NQG):
        for hp in range(HL // 2):
            sched.need(("q", hp, qg // 2, qg % 2))
            for scg in range(2):
                for hf in range(2):
                    if 2 * scg + hf <= qg:
                        sched.need(("k", hp, scg, hf))
            sched.need(("v", min(4 * qg + 3, NST - 1)))
            attn(hp, qg)

    while pend:
        emit_av(pend.pop(0))
    sched.drain_all()
    for st in (12, 13, 14, 15):
        sched.add(("o", st), o_steps(st))
    sched.drain_all()


def _build():
    nc = bacc.Bacc("TRN2", target_bir_lowering=False, debug=False,
                   num_devices=NCORES)
    xT = nc.dram_tensor("xT", [D, S], BF16, kind="ExternalInput").ap()
    wqT = nc.dram_tensor("wqT", [D, E], BF16, kind="ExternalInput").ap()
    wkT = nc.dram_tensor("wkT", [D, E], BF16, kind="ExternalInput").ap()
    wvT = nc.dram_tensor("wvT", [D, E], BF16, kind="ExternalInput").ap()
    woT = nc.dram_tensor("woT", [E, D], BF16, kind="ExternalInput").ap()
    tri = nc.dram_tensor("tri", [128, 128], BF16, kind="ExternalInput").ap()
    y = nc.dram_tensor("y", [S, D], BF16, kind="ExternalOutput").ap()
    with tile.TileContext(nc) as tc:
        _mhsa_kernel(tc, y, xT, wqT, wkT, wvT, woT, tri)
    nc.compile()
    return nc


def get_compiled():
    global _compiled
    if _compiled is None:
        _compiled = _build()
    return _compiled


def _make_tri():
    # tri[k, q] keeps key k <= query q within a 128x128 diagonal block
    k = np.arange(128)
    return (k[None, :] >= k[:, None]).astype(np.float32).astype(bf16)


def make_in_maps(inputs):
    x = np.asarray(inputs["in_features"], dtype=np.float32)
    w_q = np.asarray(inputs["w_q"], dtype=np.float32)
    w_k = np.asarray(inputs["w_k"], dtype=np.float32)
    w_v = np.asarray(inputs["w_v"], dtype=np.float32)
    w_o = np.asarray(inputs["w_o"], dtype=np.float32)
    tri = _make_tri()
    in_maps = []
    for c in range(NCORES):
        b, hg = divmod(c, 2)
        es = slice(hg * E, (hg + 1) * E)
        in_maps.append({
            "xT": x[b].T.astype(bf16),
            "wqT": w_q[es, :].T.astype(bf16),
            "wkT": w_k[es, :].T.astype(bf16),
            "wvT": w_v[es, :].T.astype(bf16),
            "woT": w_o[:, es].T.astype(bf16),
            "tri": tri,
        })
    return in_maps


def kernel(**inputs):
    global last_results
    nc = get_compiled()
    in_maps = make_in_maps(inputs)
    res = run_bass_kernel_spmd(nc, in_maps, list(range(NCORES)))
    last_results = res
    y = np.zeros((B, S, D), dtype=np.float32)
    for c in range(NCORES):
        y[c // 2] += np.asarray(res.results[c]["y"], dtype=np.float32)
    return y


# revision 18
# speedup vs baseline: 1.0190x; 1.0190x over previous
"""Multi-head self-attention (causal) Trainium2 Bass/Tile kernel, 8-way SPMD.

Sharding: data-parallel over batch (4) x tensor-parallel over heads (2 groups
of 8 heads).  Core c handles batch c//2, head-group c%2.  Each core computes
q/k/v projections for its 512 local features, causal attention for its 8
heads, and a partial o-projection (contraction over its 512 features of the
attention output) giving a full-shape [S, D] partial (bf16) that the host
sums per batch pair.

All matmul operands are bf16 (fp32 PSUM accumulation); softmax runs without
max-subtraction (scores ~ N(0,1) after the 1/8 scale, no overflow risk), with
exp on the scalar engine and the row-sum folded into the AV matmul via a ones
column appended to V.  Host pre-transposes inputs so no on-chip transposes
are needed:
  qT[e,s]  = wqT.T @ xT        (lhsT=wqT[d,e], rhs=xT[d,s])
  scoresT[sk,sq] = kT.T @ qT   (lhsT=kT[dk,sk], rhs=qT[dk,sq], K=64)
  avT[dk+1,sq]   = vaug.T @ expT  (lhsT=vaug[sk,65], rhs=expT[sk,sq])
  y[s,e]   = outT.T @ woT      (lhsT=outT[d,s], rhs=woT[d,e])

The attention inner loop is ACT(exp)-cadence-limited (~1.1us per key tile vs
~0.8us of PE fill), so all projection / o-projection matmuls are DRIPPED one
instruction at a time between attention matmuls by a tiny work-queue
scheduler; slot boundaries force-drain whatever an upcoming slot depends on.

Causal structure: off-diagonal key tiles are computed full-width; the 4
diagonal-strip tiles of each query group only compute queries >= the key
tile's start (narrowed scores / exp / AV), with a single shared [128,128]
upper-triangular mask applied to the one partially-masked 128-query chunk.
PSUM per-element has_written semantics make the narrowed AV accumulation
correct in any order (first matmul start=True pending-zeroes the bank).
"""

from contextlib import ExitStack

import numpy as np
import ml_dtypes

import concourse.bass as bass
import concourse.tile as tile
from concourse import bacc, mybir
from concourse._compat import with_exitstack
from concourse.bass_utils import run_bass_kernel_spmd

B, S, D, H = 4, 2048, 1024, 16
DK = D // H          # 64
E = 512              # local features per core (8 heads)
HL = 8               # local heads
NCORES = 8
NDT = D // 128       # 8 d-tiles
NET = E // 128       # 4 e-tiles
NST = S // 128       # 16 s-tiles
NQG = S // 512       # 4 query groups

LAG = 2              # exp -> AV pipeline lag (in key tiles)
N_WARM = 8           # HAM warm-up matmuls
DRIP_RATE = 1.2      # drip steps per off-diagonal key tile

F32 = mybir.dt.float32
BF16 = mybir.dt.bfloat16
bf16 = ml_dtypes.bfloat16

_compiled = None
last_results = None  # test harness introspection


class Drip:
    """FIFO work queue of single-matmul steps, dripped between attention
    matmuls at a budgeted rate; need() force-drains through a keyed item."""

    def __init__(self):
        self.q = []          # list of [key, steps]
        self.done = set()
        self.budget = 0.0

    def add(self, key, steps):
        self.q.append([key, list(steps)])

    def _run_one(self):
        while self.q and not self.q[0][1]:
            self.done.add(self.q.pop(0)[0])
        if not self.q:
            return False
        self.q[0][1].pop(0)()
        if not self.q[0][1]:
            self.done.add(self.q.pop(0)[0])
        return True

    def need(self, key):
        if key in self.done:
            return
        keys = [k for k, _ in self.q]
        assert key in keys, f"drip: {key} not queued"
        while key not in self.done:
            assert self._run_one()

    def drip(self, rate):
        self.budget += rate
        while self.budget >= 1.0:
            if not self._run_one():
                self.budget = 0.0
                return
            self.budget -= 1.0

    def drain_all(self):
        while self._run_one():
            pass


@with_exitstack
def _mhsa_kernel(ctx: ExitStack, tc: tile.TileContext, y, xT, wqT, wkT, wvT,
                 woT, tri):
    nc = tc.nc

    consts = ctx.enter_context(tc.tile_pool(name="consts", bufs=1))
    ex_pool = ctx.enter_context(tc.tile_pool(name="ex", bufs=10))
    rec_pool = ctx.enter_context(tc.tile_pool(name="rec", bufs=2))
    y_pool = ctx.enter_context(tc.tile_pool(name="ysb", bufs=3))
    sc_pool = ctx.enter_context(tc.tile_pool(name="sc", bufs=2, space="PSUM"))
    pj_pool = ctx.enter_context(tc.tile_pool(name="pj", bufs=2, space="PSUM"))
    av_pool = ctx.enter_context(tc.tile_pool(name="av", bufs=2, space="PSUM"))

    def ctile(shape, dt_, tg):
        return consts.tile(shape, dt_, tag=tg, name=tg)

    # ---- persistent SBUF tiles -------------------------------------------
    xT_t = ctile([128, NDT * S], BF16, "xTt")          # [p, dt*2048 + s]
    wqT_t = ctile([128, NDT * E], BF16, "wqTt")        # [p, dt*512 + e]
    wkT_t = ctile([128, NDT * E], BF16, "wkTt")
    wvT_t = ctile([128, NDT * E], BF16, "wvTt")
    woT_t = ctile([128, NET * D], BF16, "woTt")        # [p, et*1024 + d]
    qT_t = [ctile([128, S], BF16, f"qT{i}") for i in range(NET)]
    kT_t = [ctile([128, S], BF16, f"kT{i}") for i in range(NET)]
    vaug_t = [ctile([128, HL * (DK + 1)], BF16, f"vaug{i}") for i in range(NST)]
    outT_t = [ctile([128, S], BF16, f"outT{i}") for i in range(NET)]
    tri_t = ctile([128, 128], BF16, "tri")
    warm = ctile([128, 512], BF16, "warm")

    # x is host-packed chunk-major so every load below is fully contiguous:
    # chunk c0 = s[0:512) at offset 0 (t-major, 512 per t), c1 = s[512:1024)
    # at 4096, c2 = s[1024:2048) at 8192 (1024 per t).  No kernel slice
    # crosses a chunk boundary (all uses are 128-aligned within a 512 chunk).
    def x_sl(dt_, a, b):
        if b <= 512:
            base, tl, off = 0, 512, a
        elif a >= 512 and b <= 1024:
            base, tl, off = 4096, 512, a - 512
        else:
            assert a >= 1024 and b <= 2048, (a, b)
            base, tl, off = 8192, 1024, a - 1024
        p = base + dt_ * tl + off
        return xT_t[:, p:p + (b - a)]

    def wq_sl(dt_, et):
        return wqT_t[:, dt_ * E + et * 128:dt_ * E + (et + 1) * 128]

    def wk_sl(dt_, et):
        return wkT_t[:, dt_ * E + et * 128:dt_ * E + (et + 1) * 128]

    def wv_sl(dt_):
        return wvT_t[:, dt_ * E:(dt_ + 1) * E]

    def wo_sl(et, hf):
        return woT_t[:, et * D + hf * 512:et * D + (hf + 1) * 512]

    # ---- input loads: all host-packed + contiguous, spread over the 3
    # DMA-capable queues.  gpsimd: x chunk c0 (feeds the first v chains);
    # sync: wvT then x chunks c1/c2; scalar: wqT, wkT, tri, woT.
    # warm tile memset on the (idle) vector queue so the HAM warm-up
    # matmuls are not stuck behind DMA issues on gpsimd.
    nc.vector.memset(warm, 0.0)

    nc.sync.dma_start(out=wvT_t, in_=wvT)
    nc.gpsimd.dma_start(out=xT_t[:, 0:4096], in_=xT[:, 0:4096])
    nc.scalar.dma_start(out=wqT_t, in_=wqT)
    nc.scalar.dma_start(out=wkT_t, in_=wkT)
    nc.sync.dma_start(out=xT_t[:, 4096:8192], in_=xT[:, 4096:8192])
    nc.sync.dma_start(out=xT_t[:, 8192:16384], in_=xT[:, 8192:16384])
    nc.scalar.dma_start(out=tri_t, in_=tri)
    nc.scalar.dma_start(out=woT_t, in_=woT)

    # ---- PE warm-up: HAM releases after ~3.4us of sustained matmuls ------
    for _ in range(N_WARM):
        wps = pj_pool.tile([128, 512], F32, tag="pj", name="wps")
        nc.tensor.matmul(wps, lhsT=warm[:, 0:128], rhs=warm,
                         start=True, stop=True)

    # ---- projection chains (single-matmul drip steps) --------------------
    def qk_steps(wsl, dst, et, scg, hf):
        steps = []
        state = {}
        for dt_ in range(NDT):
            def step(dt_=dt_):
                if dt_ == 0:
                    state[0] = pj_pool.tile([128, 512], F32, tag="pj",
                                            name="pj")
                ps = state[0]
                s0 = scg * 1024 + hf * 512
                nc.tensor.matmul(
                    ps, lhsT=wsl(dt_, et), rhs=x_sl(dt_, s0, s0 + 512),
                    start=(dt_ == 0), stop=(dt_ == NDT - 1))
                if dt_ == NDT - 1:
                    nc.vector.tensor_copy(dst[et][:, s0:s0 + 512], ps)
            steps.append(step)
        return steps

    def v_steps(st):
        steps = []
        state = {}
        for dt_ in range(NDT):
            def step(dt_=dt_):
                if dt_ == 0:
                    state[0] = pj_pool.tile([128, 512], F32, tag="pj",
                                            name="pj")
                ps = state[0]
                nc.tensor.matmul(
                    ps, lhsT=x_sl(dt_, st * 128, (st + 1) * 128), rhs=wv_sl(dt_),
                    start=(dt_ == 0), stop=(dt_ == NDT - 1))
                if dt_ == NDT - 1:
                    nc.vector.memset(vaug_t[st], 1.0)
                    nc.vector.tensor_copy(
                        vaug_t[st].rearrange("p (h c) -> p h c", c=65)[:, :, 0:64],
                        ps.rearrange("p (h c) -> p h c", c=64))
            steps.append(step)
        return steps

    def o_steps(st):
        steps = []
        state = {}
        for hf in range(2):
            for et in range(NET):
                def step(hf=hf, et=et, st=st):
                    if et == 0:
                        state[hf] = pj_pool.tile([128, 512], F32, tag="pj",
                                                 name="pj")
                    ps = state[hf]
                    nc.tensor.matmul(
                        ps, lhsT=outT_t[et][:, st * 128:(st + 1) * 128],
                        rhs=wo_sl(et, hf),
                        start=(et == 0), stop=(et == NET - 1))
                    if et == NET - 1:
                        ysb = y_pool.tile([128, 512], BF16, tag="ysb",
                                          name="ysb")
                        nc.vector.tensor_copy(ysb, ps)
                        nc.sync.dma_start(
                            out=y[st * 128:(st + 1) * 128,
                                  hf * 512:(hf + 1) * 512],
                            in_=ysb)
                steps.append(step)
        return steps

    # Softmax denominators bounce through DRAM: DVE can only write at
    # 32-aligned base partitions, and SBUF APs cannot have a step-0
    # partition dim (needed for the broadcast) — DRAM APs can do both.
    sums_dram = nc.dram_tensor("sums_bounce", [NQG, HL, 512], F32).ap()
    rec_dram = nc.dram_tensor("rec_bounce", [NQG, HL, 512], BF16).ap()

    # ones2: selector for the final pair's reciprocal broadcast matmul —
    # bc[j, :] = recb2[0, :] for j<64 (head A) and recb2[32, :] for j>=64
    # (head B).  Rows 0/32 because the DVE can only write at 32-aligned
    # partitions; K padded to 64 (a K=33 matmul wedges the exec unit).
    ones2 = ctile([64, 128], BF16, "ones2")
    nc.vector.memset(ones2, 0.0)
    nc.vector.memset(ones2[0:1, 0:64], 1.0)
    nc.vector.memset(ones2[32:33, 64:128], 1.0)

    sched = Drip()

    # ---- attention: one global pipeline over all (qg, hp, kt) -------------
    # Heads hA=2*hp (partitions 0:64) and hB=2*hp+1 (64:128) share each
    # score tile: [:, 0:512]=A, [:, 512:1024]=B for one key tile kt.  The
    # K=64 score matmuls for A and B land on disjoint PE row groups (base
    # partition 0 vs 64) and run concurrently.  outT stays UNNORMALIZED;
    # denominators are collected and normalization is batched per qg.
    # Diagonal-strip tiles (kt-4*qg = j >= 0) are narrowed to queries
    # >= 128*j within the group.  The exp->AV lag spans slot boundaries so
    # the PE never waits on a fresh exp at a slot transition; a slot's
    # stash (and its boundary actions) runs when its last AV pops, ~LAG
    # key tiles into the next slot.
    pend = []

    def emit_av(it):
        kt, ex, qo, avA, avB, qg, hp = it
        nk = 4 * qg + 4
        for av, h in ((avA, 2 * hp), (avB, 2 * hp + 1)):
            nc.tensor.matmul(
                av[:, qo:512],
                lhsT=vaug_t[kt][:, h * 65:h * 65 + 65],
                rhs=ex[:, (h & 1) * 512 + qo:((h & 1) + 1) * 512],
                start=(kt == 0), stop=(kt == nk - 1),
            )
        if kt == nk - 1:
            _stash(hp, qg, hp, avA, avB)
            boundary(qg, hp)

    def attn(hp, qg):
        ti = hp
        nk = 4 * qg + 4
        avA = av_pool.tile([65, 512], F32, tag="av", name="avA")
        avB = av_pool.tile([65, 512], F32, tag="av", name="avB")
        for kt in range(nk):
            j = kt - 4 * qg          # >=0 on the diagonal strip
            qo = 128 * j if j > 0 else 0
            ps = sc_pool.tile([128, 1024], F32, tag="sc", name="ps")
            for po in (0, 64):
                nc.tensor.matmul(
                    ps[:, (po // 64) * 512 + qo:(po // 64 + 1) * 512],
                    lhsT=kT_t[ti][po:po + 64, kt * 128:(kt + 1) * 128],
                    rhs=qT_t[ti][po:po + 64, qg * 512 + qo:(qg + 1) * 512],
                    start=True, stop=True,
                )
            ex = ex_pool.tile([128, 1024], BF16, tag="ex", name="ex")
            if qo == 0:
                nc.scalar.activation(out=ex, in_=ps,
                                     func=mybir.ActivationFunctionType.Exp,
                                     scale=0.125)
            else:
                for half in range(2):
                    a = half * 512 + qo
                    b = (half + 1) * 512
                    nc.scalar.activation(
                        out=ex[:, a:b], in_=ps[:, a:b],
                        func=mybir.ActivationFunctionType.Exp, scale=0.125)
            if j >= 0:  # triangular mask on the one partial 128-q chunk
                for half in range(2):
                    a = half * 512 + qo
                    nc.vector.tensor_mul(ex[:, a:a + 128], ex[:, a:a + 128],
                                         tri_t)
            pend.append((kt, ex, qo, avA, avB, qg, hp))
            rate = DRIP_RATE if j < 1 else max(0.2, DRIP_RATE - 0.35 * j)
            sched.drip(rate)
            while len(pend) > LAG:
                emit_av(pend.pop(0))

    def _stash(hp, qg, ti, avA, avB):
        # stash unnormalized outputs + denominators; release av quickly
        hA, hB = 2 * hp, 2 * hp + 1
        if qg == NQG - 1 and hp == HL // 2 - 1:
            # final pair: no attention left to hide the DRAM-bounce latency
            # behind, so normalize inline via reciprocal + PE broadcast
            stg2 = rec_pool.tile([64, 512], F32, tag="stg2", name="stg2")
            nc.vector.memset(stg2, 1.0)
            for av, po, row in ((avA, 0, 0), (avB, 64, 32)):
                nc.vector.tensor_copy(
                    outT_t[ti][po:po + 64, qg * 512:(qg + 1) * 512],
                    av[0:64, :])
                nc.vector.tensor_copy(stg2[row:row + 1, :], av[64:65, :])
            rec2 = rec_pool.tile([64, 512], F32, tag="rec2", name="rec2")
            nc.vector.reciprocal_approx_fast(out=rec2, in_=stg2)
            recb2 = rec_pool.tile([64, 512], BF16, tag="recb2", name="recb2")
            nc.vector.tensor_copy(recb2, rec2)
            bc = av_pool.tile([128, 512], F32, tag="av", name="bc")
            nc.tensor.matmul(bc, lhsT=ones2, rhs=recb2, start=True, stop=True)
            for po in (0, 64):
                sl = outT_t[ti][po:po + 64, qg * 512:(qg + 1) * 512]
                # bc lives in PSUM, which gpsimd cannot read — keep on DVE
                nc.vector.tensor_mul(sl, sl, bc[po:po + 64, :])
        else:
            for av, h, po in ((avA, hA, 0), (avB, hB, 64)):
                nc.vector.tensor_copy(
                    outT_t[ti][po:po + 64, qg * 512:(qg + 1) * 512],
                    av[0:64, :])
                stg = rec_pool.tile([1, 512], F32, tag="stg", name="stg",
                                    bufs=4)
                nc.vector.tensor_copy(stg, av[64:65, :])
                nc.sync.dma_start(out=sums_dram[qg, h], in_=stg)

    # ---- batched normalization (DRAM-bounce broadcast) -------------------
    def _norm_heads(qg, heads):
        h0, nh = heads[0], len(heads)
        sums = rec_pool.tile([nh, 512], F32, tag=f"sums{nh}", name="sums")
        nc.sync.dma_start(out=sums, in_=sums_dram[qg, h0:h0 + nh])
        rec = rec_pool.tile([nh, 512], F32, tag=f"rec{nh}", name="rec")
        nc.vector.reciprocal_approx_fast(out=rec, in_=sums)
        recb = rec_pool.tile([nh, 512], BF16, tag=f"recb{nh}", name="recb")
        nc.vector.tensor_copy(recb, rec)
        nc.sync.dma_start(out=rec_dram[qg, h0:h0 + nh], in_=recb)
        for h in heads:
            ti, po = h // 2, 64 * (h % 2)
            # walrus requires SBUF tensor_tensor inputs to share the start
            # partition, so land the broadcast at the same partition range
            bcs = rec_pool.tile([128, 512], BF16, tag="bcs", name="bcs")
            nc.sync.dma_start(
                out=bcs[po:po + 64, :],
                in_=rec_dram[qg, h:h + 1, :].to_broadcast([64, 512]))
            sl = outT_t[ti][po:po + 64, qg * 512:(qg + 1) * 512]
            nc.gpsimd.tensor_mul(sl, sl, bcs[po:po + 64, :])

    def normalize(qg):
        _norm_heads(qg, list(range(HL)))

    def normalize_pair(qg, hp):
        _norm_heads(qg, [2 * hp, 2 * hp + 1])

    # post-slot boundary actions, keyed by (qg, hp) whose stash just ran
    def boundary(qg, hp):
        if qg == 1 and hp == 0:
            normalize(0)
            for st in (0, 1, 2, 3):
                sched.add(("o", st), o_steps(st))
        elif qg == 2 and hp == 0:
            normalize(1)
            for st in (4, 5, 6, 7):
                sched.add(("o", st), o_steps(st))
        elif qg == 3 and hp == 0:
            normalize(2)
            for st in (8, 9, 10, 11):
                sched.add(("o", st), o_steps(st))
        if qg == 3 and hp < 3:
            normalize_pair(3, hp)

    # ---- program order ----------------------------------------------------
    # Pre-loop: everything slot (0,0) depends on.  The drip queue holds the
    # remaining projections in the order later slots need them; oproj steps
    # are appended once their query group is normalized.
    def addq(et, scg, hf):
        sched.add(("q", et, scg, hf), qk_steps(wq_sl, qT_t, et, scg, hf))

    def addk(et, scg, hf):
        sched.add(("k", et, scg, hf), qk_steps(wk_sl, kT_t, et, scg, hf))

    for st in range(4):
        sched.add(("v", st), v_steps(st))
    addq(0, 0, 0)
    addk(0, 0, 0)
    sched.need(("v", 3))
    sched.need(("q", 0, 0, 0))
    sched.need(("k", 0, 0, 0))

    for et in (1, 2, 3):
        addq(et, 0, 0)
        addk(et, 0, 0)
    for st in (4, 5, 6, 7):
        sched.add(("v", st), v_steps(st))
    for et in (0, 1, 2, 3):
        addk(et, 0, 1)
        addq(et, 0, 1)
    for st in (8, 9, 10, 11):
        sched.add(("v", st), v_steps(st))
    for et in (0, 1, 2, 3):
        addq(et, 1, 0)
        addk(et, 1, 0)
    for st in (12, 13, 14, 15):
        sched.add(("v", st), v_steps(st))
    for et in (0, 1, 2, 3):
        addk(et, 1, 1)
        addq(et, 1, 1)

    for qg in range(NQG):
        for hp in range(HL // 2):
            sched.need(("q", hp, qg // 2, qg % 2))
            for scg in range(2):
                for hf in range(2):
                    if 2 * scg + hf <= qg:
                        sched.need(("k", hp, scg, hf))
            sched.need(("v", min(4 * qg + 3, NST - 1)))
            attn(hp, qg)

    while pend:
        emit_av(pend.pop(0))
    sched.drain_all()
    for st in (12, 13, 14, 15):
        sched.add(("o", st), o_steps(st))
    sched.drain_all()


def _build():
    nc = bacc.Bacc("TRN2", target_bir_lowering=False, debug=False,
                   num_devices=NCORES)
    # All inputs host-packed to the exact SBUF tile layouts (partition-major,
    # fully contiguous per partition) so every load DMA runs at full rate.
    xT = nc.dram_tensor("xT", [128, NDT * S], BF16, kind="ExternalInput").ap()
    wqT = nc.dram_tensor("wqT", [128, NDT * E], BF16, kind="ExternalInput").ap()
    wkT = nc.dram_tensor("wkT", [128, NDT * E], BF16, kind="ExternalInput").ap()
    wvT = nc.dram_tensor("wvT", [128, NDT * E], BF16, kind="ExternalInput").ap()
    woT = nc.dram_tensor("woT", [128, NET * D], BF16, kind="ExternalInput").ap()
    tri = nc.dram_tensor("tri", [128, 128], BF16, kind="ExternalInput").ap()
    y = nc.dram_tensor("y", [S, D], BF16, kind="ExternalOutput").ap()
    with tile.TileContext(nc) as tc:
        _mhsa_kernel(tc, y, xT, wqT, wkT, wvT, woT, tri)
    nc.compile()
    return nc


def get_compiled():
    global _compiled
    if _compiled is None:
        _compiled = _build()
    return _compiled


def _make_tri():
    # tri[k, q] keeps key k <= query q within a 128x128 diagonal block
    k = np.arange(128)
    return (k[None, :] >= k[:, None]).astype(np.float32).astype(bf16)


def _pack_dtiles(m):
    # [T*128, F] -> [128, T*F]: d-tile-major per partition row
    t = m.shape[0] // 128
    return np.ascontiguousarray(
        m.reshape(t, 128, m.shape[1]).transpose(1, 0, 2).reshape(128, -1))


def _pack_x(xc):
    # [1024, 2048] -> [128, 16384] chunk-major: s[0:512) | s[512:1024) |
    # s[1024:2048), each chunk d-tile-major (matches kernel x_sl layout)
    return np.concatenate([
        _pack_dtiles(np.ascontiguousarray(xc[:, 0:512])),
        _pack_dtiles(np.ascontiguousarray(xc[:, 512:1024])),
        _pack_dtiles(np.ascontiguousarray(xc[:, 1024:2048])),
    ], axis=1)


def make_in_maps(inputs):
    x = np.asarray(inputs["in_features"], dtype=np.float32)
    w_q = np.asarray(inputs["w_q"], dtype=np.float32)
    w_k = np.asarray(inputs["w_k"], dtype=np.float32)
    w_v = np.asarray(inputs["w_v"], dtype=np.float32)
    w_o = np.asarray(inputs["w_o"], dtype=np.float32)
    tri = _make_tri()
    in_maps = []
    for c in range(NCORES):
        b, hg = divmod(c, 2)
        es = slice(hg * E, (hg + 1) * E)
        in_maps.append({
            "xT": _pack_x(x[b].T).astype(bf16),
            "wqT": _pack_dtiles(w_q[es, :].T).astype(bf16),
            "wkT": _pack_dtiles(w_k[es, :].T).astype(bf16),
            "wvT": _pack_dtiles(w_v[es, :].T).astype(bf16),
            "woT": _pack_dtiles(w_o[:, es].T).astype(bf16),
            "tri": tri,
        })
    return in_maps


def kernel(**inputs):
    global last_results
    nc = get_compiled()
    in_maps = make_in_maps(inputs)
    res = run_bass_kernel_spmd(nc, in_maps, list(range(NCORES)))
    last_results = res
    y = np.zeros((B, S, D), dtype=np.float32)
    for c in range(NCORES):
        y[c // 2] += np.asarray(res.results[c]["y"], dtype=np.float32)
    return y


# revision 23
# speedup vs baseline: 1.0413x; 1.0219x over previous
"""Multi-head self-attention (causal) Trainium2 Bass/Tile kernel, 8-way SPMD.

Sharding: data-parallel over batch (4) x tensor-parallel over heads (2 groups
of 8 heads).  Core c handles batch c//2, head-group c%2.  Each core computes
q/k/v projections for its 512 local features, causal attention for its 8
heads, and a partial o-projection (contraction over its 512 features of the
attention output) giving a full-shape [S, D] partial (bf16) that the host
sums per batch pair.

All matmul operands are bf16 (fp32 PSUM accumulation); softmax runs without
max-subtraction (scores ~ N(0,1) after the 1/8 scale, no overflow risk), with
exp on the scalar engine and the row-sum folded into the AV matmul via a ones
column appended to V.  Host pre-transposes inputs so no on-chip transposes
are needed:
  qT[e,s]  = wqT.T @ xT        (lhsT=wqT[d,e], rhs=xT[d,s])
  scoresT[sk,sq] = kT.T @ qT   (lhsT=kT[dk,sk], rhs=qT[dk,sq], K=64)
  avT[dk+1,sq]   = vaug.T @ expT  (lhsT=vaug[sk,65], rhs=expT[sk,sq])
  y[s,e]   = outT.T @ woT      (lhsT=outT[d,s], rhs=woT[d,e])

The attention inner loop is ACT(exp)-cadence-limited (~1.1us per key tile vs
~0.8us of PE fill), so all projection / o-projection matmuls are DRIPPED one
instruction at a time between attention matmuls by a tiny work-queue
scheduler; slot boundaries force-drain whatever an upcoming slot depends on.

Causal structure: off-diagonal key tiles are computed full-width; the 4
diagonal-strip tiles of each query group only compute queries >= the key
tile's start (narrowed scores / exp / AV), with a single shared [128,128]
upper-triangular mask applied to the one partially-masked 128-query chunk.
PSUM per-element has_written semantics make the narrowed AV accumulation
correct in any order (first matmul start=True pending-zeroes the bank).
"""

from contextlib import ExitStack

import numpy as np
import ml_dtypes

import concourse.bass as bass
import concourse.tile as tile
from concourse import bacc, mybir
from concourse._compat import with_exitstack
from concourse.bass_utils import run_bass_kernel_spmd

B, S, D, H = 4, 2048, 1024, 16
DK = D // H          # 64
E = 512              # local features per core (8 heads)
HL = 8               # local heads
NCORES = 8
NDT = D // 128       # 8 d-tiles
NET = E // 128       # 4 e-tiles
NST = S // 128       # 16 s-tiles
NQG = S // 512       # 4 query groups

LAG = 2              # exp -> AV pipeline lag (in key tiles)
N_WARM = 8           # HAM warm-up matmuls
DRIP_RATE = 1.2      # drip steps per off-diagonal key tile

F32 = mybir.dt.float32
BF16 = mybir.dt.bfloat16
bf16 = ml_dtypes.bfloat16

_compiled = None
last_results = None  # test harness introspection


class Drip:
    """FIFO work queue of single-matmul steps, dripped between attention
    matmuls at a budgeted rate; need() force-drains through a keyed item."""

    def __init__(self):
        self.q = []          # list of [key, steps]
        self.done = set()
        self.budget = 0.0

    def add(self, key, steps):
        self.q.append([key, list(steps)])

    def _run_one(self):
        while self.q and not self.q[0][1]:
            self.done.add(self.q.pop(0)[0])
        if not self.q:
            return False
        self.q[0][1].pop(0)()
        if not self.q[0][1]:
            self.done.add(self.q.pop(0)[0])
        return True

    def need(self, key):
        if key in self.done:
            return
        keys = [k for k, _ in self.q]
        assert key in keys, f"drip: {key} not queued"
        while key not in self.done:
            assert self._run_one()

    def drip(self, rate):
        self.budget += rate
        while self.budget >= 1.0:
            if not self._run_one():
                self.budget = 0.0
                return
            self.budget -= 1.0

    def drain_all(self):
        while self._run_one():
            pass


@with_exitstack
def _mhsa_kernel(ctx: ExitStack, tc: tile.TileContext, y, xT, wqT, wkT, wvT,
                 woT, tri):
    nc = tc.nc

    consts = ctx.enter_context(tc.tile_pool(name="consts", bufs=1))
    ex_pool = ctx.enter_context(tc.tile_pool(name="ex", bufs=10))
    rec_pool = ctx.enter_context(tc.tile_pool(name="rec", bufs=2))
    y_pool = ctx.enter_context(tc.tile_pool(name="ysb", bufs=3))
    sc_pool = ctx.enter_context(tc.tile_pool(name="sc", bufs=2, space="PSUM"))
    pj_pool = ctx.enter_context(tc.tile_pool(name="pj", bufs=2, space="PSUM"))
    av_pool = ctx.enter_context(tc.tile_pool(name="av", bufs=2, space="PSUM"))

    def ctile(shape, dt_, tg):
        return consts.tile(shape, dt_, tag=tg, name=tg)

    # ---- persistent SBUF tiles -------------------------------------------
    xT_t = ctile([128, NDT * S], BF16, "xTt")          # [p, dt*2048 + s]
    wqT_t = ctile([128, NDT * E], BF16, "wqTt")        # [p, dt*512 + e]
    wkT_t = ctile([128, NDT * E], BF16, "wkTt")
    wvT_t = ctile([128, NDT * E], BF16, "wvTt")
    woT_t = ctile([128, NET * D], BF16, "woTt")        # [p, et*1024 + d]
    qT_t = [ctile([128, S], BF16, f"qT{i}") for i in range(NET)]
    kT_t = [ctile([128, S], BF16, f"kT{i}") for i in range(NET)]
    vaug_t = [ctile([128, HL * (DK + 1)], BF16, f"vaug{i}") for i in range(NST)]
    outT_t = [ctile([128, S], BF16, f"outT{i}") for i in range(NET)]
    tri_t = ctile([128, 128], BF16, "tri")
    warm = ctile([128, 512], BF16, "warm")

    # x is host-packed chunk-major so every load below is fully contiguous:
    # chunk c0 = s[0:512) at offset 0 (t-major, 512 per t), c1 = s[512:1024)
    # at 4096, c2 = s[1024:2048) at 8192 (1024 per t).  No kernel slice
    # crosses a chunk boundary (all uses are 128-aligned within a 512 chunk).
    def x_sl(dt_, a, b):
        if b <= 512:
            base, tl, off = 0, 512, a
        elif a >= 512 and b <= 1024:
            base, tl, off = 4096, 512, a - 512
        else:
            assert a >= 1024 and b <= 2048, (a, b)
            base, tl, off = 8192, 1024, a - 1024
        p = base + dt_ * tl + off
        return xT_t[:, p:p + (b - a)]

    def wq_sl(dt_, et):
        return wqT_t[:, dt_ * E + et * 128:dt_ * E + (et + 1) * 128]

    def wk_sl(dt_, et):
        return wkT_t[:, dt_ * E + et * 128:dt_ * E + (et + 1) * 128]

    def wv_sl(dt_):
        return wvT_t[:, dt_ * E:(dt_ + 1) * E]

    def wo_sl(et, hf):
        return woT_t[:, et * D + hf * 512:et * D + (hf + 1) * 512]

    # ---- input loads: all host-packed + contiguous.  DMA engines stripe
    # every transfer across 16 engines sharing ~350GB/s, so what matters is
    # PHASING: only first-needed bytes in flight early.  Phase 1: c0 (gpsimd)
    # + wv..c1 FIFO (sync) + tri (scalar).  x chunk c2 and woT (3MB, needed
    # from ~40us) are issued later from boundary(0,3) to keep them from
    # stealing phase-1 bandwidth.
    # warm tile memset on the (idle) vector queue so the HAM warm-up
    # matmuls are not stuck behind DMA issues on gpsimd.
    nc.vector.memset(warm, 0.0)

    nc.gpsimd.dma_start(out=xT_t[:, 0:4096], in_=xT[:, 0:4096])
    nc.sync.dma_start(out=wvT_t, in_=wvT)
    nc.sync.dma_start(out=wqT_t, in_=wqT)
    nc.sync.dma_start(out=wkT_t, in_=wkT)
    nc.sync.dma_start(out=xT_t[:, 4096:8192], in_=xT[:, 4096:8192])
    nc.scalar.dma_start(out=tri_t, in_=tri)

    def load_phase2():
        nc.gpsimd.dma_start(out=xT_t[:, 8192:16384], in_=xT[:, 8192:16384])
        nc.gpsimd.dma_start(out=woT_t, in_=woT)

    # ---- PE warm-up: HAM releases after ~3.4us of sustained matmuls ------
    for _ in range(N_WARM):
        wps = pj_pool.tile([128, 512], F32, tag="pj", name="wps")
        nc.tensor.matmul(wps, lhsT=warm[:, 0:128], rhs=warm,
                         start=True, stop=True)

    # ---- projection chains (single-matmul drip steps) --------------------
    def qk_steps(wsl, dst, et, scg, hf):
        steps = []
        state = {}
        for dt_ in range(NDT):
            def step(dt_=dt_):
                if dt_ == 0:
                    state[0] = pj_pool.tile([128, 512], F32, tag="pj",
                                            name="pj")
                ps = state[0]
                s0 = scg * 1024 + hf * 512
                nc.tensor.matmul(
                    ps, lhsT=wsl(dt_, et), rhs=x_sl(dt_, s0, s0 + 512),
                    start=(dt_ == 0), stop=(dt_ == NDT - 1))
                if dt_ == NDT - 1:
                    nc.vector.tensor_copy(dst[et][:, s0:s0 + 512], ps)
            steps.append(step)
        return steps

    def v_steps(st):
        steps = []
        state = {}
        for dt_ in range(NDT):
            def step(dt_=dt_):
                if dt_ == 0:
                    state[0] = pj_pool.tile([128, 512], F32, tag="pj",
                                            name="pj")
                ps = state[0]
                nc.tensor.matmul(
                    ps, lhsT=x_sl(dt_, st * 128, (st + 1) * 128), rhs=wv_sl(dt_),
                    start=(dt_ == 0), stop=(dt_ == NDT - 1))
                if dt_ == NDT - 1:
                    nc.vector.memset(vaug_t[st], 1.0)
                    nc.vector.tensor_copy(
                        vaug_t[st].rearrange("p (h c) -> p h c", c=65)[:, :, 0:64],
                        ps.rearrange("p (h c) -> p h c", c=64))
            steps.append(step)
        return steps

    def o_steps(st):
        steps = []
        state = {}
        for hf in range(2):
            for et in range(NET):
                def step(hf=hf, et=et, st=st):
                    if et == 0:
                        state[hf] = pj_pool.tile([128, 512], F32, tag="pj",
                                                 name="pj")
                    ps = state[hf]
                    nc.tensor.matmul(
                        ps, lhsT=outT_t[et][:, st * 128:(st + 1) * 128],
                        rhs=wo_sl(et, hf),
                        start=(et == 0), stop=(et == NET - 1))
                    if et == NET - 1:
                        ysb = y_pool.tile([128, 512], BF16, tag="ysb",
                                          name="ysb")
                        nc.vector.tensor_copy(ysb, ps)
                        nc.sync.dma_start(
                            out=y[st * 128:(st + 1) * 128,
                                  hf * 512:(hf + 1) * 512],
                            in_=ysb)
                steps.append(step)
        return steps

    # Softmax denominators bounce through DRAM: DVE can only write at
    # 32-aligned base partitions, and SBUF APs cannot have a step-0
    # partition dim (needed for the broadcast) — DRAM APs can do both.
    sums_dram = nc.dram_tensor("sums_bounce", [NQG, HL, 512], F32).ap()
    rec_dram = nc.dram_tensor("rec_bounce", [NQG, HL, 512], BF16).ap()

    # ones2: selector for the final pair's reciprocal broadcast matmul —
    # bc[j, :] = recb2[0, :] for j<64 (head A) and recb2[32, :] for j>=64
    # (head B).  Rows 0/32 because the DVE can only write at 32-aligned
    # partitions; K padded to 64 (a K=33 matmul wedges the exec unit).
    ones2 = ctile([64, 128], BF16, "ones2")
    nc.vector.memset(ones2, 0.0)
    nc.vector.memset(ones2[0:1, 0:64], 1.0)
    nc.vector.memset(ones2[32:33, 64:128], 1.0)

    sched = Drip()

    # ---- attention: one global pipeline over all (qg, hp, kt) -------------
    # Heads hA=2*hp (partitions 0:64) and hB=2*hp+1 (64:128) share each
    # score tile: [:, 0:512]=A, [:, 512:1024]=B for one key tile kt.  The
    # K=64 score matmuls for A and B land on disjoint PE row groups (base
    # partition 0 vs 64) and run concurrently.  outT stays UNNORMALIZED;
    # denominators are collected and normalization is batched per qg.
    # Diagonal-strip tiles (kt-4*qg = j >= 0) are narrowed to queries
    # >= 128*j within the group.  The exp->AV lag spans slot boundaries so
    # the PE never waits on a fresh exp at a slot transition; a slot's
    # stash (and its boundary actions) runs when its last AV pops, ~LAG
    # key tiles into the next slot.
    pend = []

    def emit_av(it):
        kt, ex, qo, avA, avB, qg, hp = it
        nk = 4 * qg + 4
        for av, h in ((avA, 2 * hp), (avB, 2 * hp + 1)):
            nc.tensor.matmul(
                av[:, qo:512],
                lhsT=vaug_t[kt][:, h * 65:h * 65 + 65],
                rhs=ex[:, (h & 1) * 512 + qo:((h & 1) + 1) * 512],
                start=(kt == 0), stop=(kt == nk - 1),
            )
        if kt == nk - 1:
            if (qg, hp) == (NQG - 1, HL // 2 - 1):
                return  # final slot: stash emitted manually in the tail
            _stash(hp, qg, hp, avA, avB)
            boundary(qg, hp)

    def attn(hp, qg):
        ti = hp
        nk = 4 * qg + 4
        avA = av_pool.tile([65, 512], F32, tag="av", name="avA")
        avB = av_pool.tile([65, 512], F32, tag="av", name="avB")
        for kt in range(nk):
            j = kt - 4 * qg          # >=0 on the diagonal strip
            qo = 128 * j if j > 0 else 0
            ps = sc_pool.tile([128, 1024], F32, tag="sc", name="ps")
            for po in (0, 64):
                nc.tensor.matmul(
                    ps[:, (po // 64) * 512 + qo:(po // 64 + 1) * 512],
                    lhsT=kT_t[ti][po:po + 64, kt * 128:(kt + 1) * 128],
                    rhs=qT_t[ti][po:po + 64, qg * 512 + qo:(qg + 1) * 512],
                    start=True, stop=True,
                )
            ex = ex_pool.tile([128, 1024], BF16, tag="ex", name="ex")
            if qo == 0:
                nc.scalar.activation(out=ex, in_=ps,
                                     func=mybir.ActivationFunctionType.Exp,
                                     scale=0.125)
            else:
                for half in range(2):
                    a = half * 512 + qo
                    b = (half + 1) * 512
                    nc.scalar.activation(
                        out=ex[:, a:b], in_=ps[:, a:b],
                        func=mybir.ActivationFunctionType.Exp, scale=0.125)
            if j >= 0:  # triangular mask on the one partial 128-q chunk
                for half in range(2):
                    a = half * 512 + qo
                    nc.vector.tensor_mul(ex[:, a:a + 128], ex[:, a:a + 128],
                                         tri_t)
            pend.append((kt, ex, qo, avA, avB, qg, hp))
            rate = DRIP_RATE if j < 1 else max(0.2, DRIP_RATE - 0.35 * j)
            sched.drip(rate)
            while len(pend) > LAG:
                emit_av(pend.pop(0))

    def _stash(hp, qg, ti, avA, avB):
        # stash unnormalized outputs + denominators; release av quickly
        hA, hB = 2 * hp, 2 * hp + 1
        if qg == NQG - 1 and hp == HL // 2 - 1:
            # final pair: no attention left to hide the DRAM-bounce latency
            # behind, so normalize inline via reciprocal + PE broadcast
            stg2 = rec_pool.tile([64, 512], F32, tag="stg2", name="stg2")
            nc.vector.memset(stg2, 1.0)
            for av, po, row in ((avA, 0, 0), (avB, 64, 32)):
                nc.vector.tensor_copy(
                    outT_t[ti][po:po + 64, qg * 512:(qg + 1) * 512],
                    av[0:64, :])
                nc.vector.tensor_copy(stg2[row:row + 1, :], av[64:65, :])
            rec2 = rec_pool.tile([64, 512], F32, tag="rec2", name="rec2")
            nc.vector.reciprocal_approx_fast(out=rec2, in_=stg2)
            recb2 = rec_pool.tile([64, 512], BF16, tag="recb2", name="recb2")
            nc.vector.tensor_copy(recb2, rec2)
            bc = av_pool.tile([128, 512], F32, tag="av", name="bc")
            nc.tensor.matmul(bc, lhsT=ones2, rhs=recb2, start=True, stop=True)
            for po in (0, 64):
                sl = outT_t[ti][po:po + 64, qg * 512:(qg + 1) * 512]
                # bc lives in PSUM, which gpsimd cannot read — keep on DVE
                nc.vector.tensor_mul(sl, sl, bc[po:po + 64, :])
        else:
            for av, h, po in ((avA, hA, 0), (avB, hB, 64)):
                nc.vector.tensor_copy(
                    outT_t[ti][po:po + 64, qg * 512:(qg + 1) * 512],
                    av[0:64, :])
                stg = rec_pool.tile([1, 512], F32, tag="stg", name="stg",
                                    bufs=4)
                nc.vector.tensor_copy(stg, av[64:65, :])
                nc.sync.dma_start(out=sums_dram[qg, h], in_=stg)

    # ---- batched normalization (DRAM-bounce broadcast) -------------------
    def _norm_heads(qg, heads):
        h0, nh = heads[0], len(heads)
        sums = rec_pool.tile([nh, 512], F32, tag=f"sums{nh}", name="sums")
        nc.sync.dma_start(out=sums, in_=sums_dram[qg, h0:h0 + nh])
        rec = rec_pool.tile([nh, 512], F32, tag=f"rec{nh}", name="rec")
        nc.vector.reciprocal_approx_fast(out=rec, in_=sums)
        recb = rec_pool.tile([nh, 512], BF16, tag=f"recb{nh}", name="recb")
        nc.vector.tensor_copy(recb, rec)
        nc.sync.dma_start(out=rec_dram[qg, h0:h0 + nh], in_=recb)
        for h in heads:
            ti, po = h // 2, 64 * (h % 2)
            # walrus requires SBUF tensor_tensor inputs to share the start
            # partition, so land the broadcast at the same partition range
            bcs = rec_pool.tile([128, 512], BF16, tag="bcs", name="bcs")
            nc.sync.dma_start(
                out=bcs[po:po + 64, :],
                in_=rec_dram[qg, h:h + 1, :].to_broadcast([64, 512]))
            sl = outT_t[ti][po:po + 64, qg * 512:(qg + 1) * 512]
            nc.gpsimd.tensor_mul(sl, sl, bcs[po:po + 64, :])

    def normalize(qg):
        _norm_heads(qg, list(range(HL)))

    def normalize_pair(qg, hp):
        _norm_heads(qg, [2 * hp, 2 * hp + 1])

    # post-slot boundary actions, keyed by (qg, hp) whose stash just ran
    def boundary(qg, hp):
        if qg == 0 and hp == 3:
            load_phase2()
        if qg == 1 and hp == 0:
            normalize(0)
            for st in (0, 1, 2, 3):
                sched.add(("o", st), o_steps(st))
        elif qg == 2 and hp == 0:
            normalize(1)
            for st in (4, 5, 6, 7):
                sched.add(("o", st), o_steps(st))
        elif qg == 3 and hp == 0:
            normalize(2)
            for st in (8, 9, 10, 11):
                sched.add(("o", st), o_steps(st))
        if qg == 3 and hp < 3:
            normalize_pair(3, hp)

    # ---- program order ----------------------------------------------------
    # Pre-loop: everything slot (0,0) depends on.  The drip queue holds the
    # remaining projections in the order later slots need them; oproj steps
    # are appended once their query group is normalized.
    def addq(et, scg, hf):
        sched.add(("q", et, scg, hf), qk_steps(wq_sl, qT_t, et, scg, hf))

    def addk(et, scg, hf):
        sched.add(("k", et, scg, hf), qk_steps(wk_sl, kT_t, et, scg, hf))

    for st in range(4):
        sched.add(("v", st), v_steps(st))
    addq(0, 0, 0)
    addk(0, 0, 0)
    sched.need(("v", 3))
    sched.need(("q", 0, 0, 0))
    sched.need(("k", 0, 0, 0))

    for et in (1, 2, 3):
        addq(et, 0, 0)
        addk(et, 0, 0)
    for st in (4, 5, 6, 7):
        sched.add(("v", st), v_steps(st))
    for et in (0, 1, 2, 3):
        addk(et, 0, 1)
        addq(et, 0, 1)
    for st in (8, 9, 10, 11):
        sched.add(("v", st), v_steps(st))
    for et in (0, 1, 2, 3):
        addq(et, 1, 0)
        addk(et, 1, 0)
    for st in (12, 13, 14, 15):
        sched.add(("v", st), v_steps(st))
    for et in (0, 1, 2, 3):
        addk(et, 1, 1)
        addq(et, 1, 1)

    for qg in range(NQG):
        for hp in range(HL // 2):
            sched.need(("q", hp, qg // 2, qg % 2))
            for scg in range(2):
                for hf in range(2):
                    if 2 * scg + hf <= qg:
                        sched.need(("k", hp, scg, hf))
            sched.need(("v", min(4 * qg + 3, NST - 1)))
            attn(hp, qg)

    # Final drain: the last two AVs of (3,3) wait on the freshest exps and
    # the stash's inline normalization is a serial DVE chain; drip queued
    # oproj steps into both wait windows so the PE stays fed, and emit the
    # final stash manually between them.
    assert len(pend) == LAG
    emit_av(pend.pop(0))
    for _ in range(5):
        sched._run_one()
    it15 = pend.pop(0)
    emit_av(it15)
    for _ in range(8):
        sched._run_one()
    _stash(HL // 2 - 1, NQG - 1, HL // 2 - 1, it15[3], it15[4])
    sched.drain_all()
    for st in (12, 13, 14, 15):
        sched.add(("o", st), o_steps(st))
    sched.drain_all()


def _build():
    nc = bacc.Bacc("TRN2", target_bir_lowering=False, debug=False,
                   num_devices=NCORES)
    # All inputs host-packed to the exact SBUF tile layouts (partition-major,
    # fully contiguous per partition) so every load DMA runs at full rate.
    xT = nc.dram_tensor("xT", [128, NDT * S], BF16, kind="ExternalInput").ap()
    wqT = nc.dram_tensor("wqT", [128, NDT * E], BF16, kind="ExternalInput").ap()
    wkT = nc.dram_tensor("wkT", [128, NDT * E], BF16, kind="ExternalInput").ap()
    wvT = nc.dram_tensor("wvT", [128, NDT * E], BF16, kind="ExternalInput").ap()
    woT = nc.dram_tensor("woT", [128, NET * D], BF16, kind="ExternalInput").ap()
    tri = nc.dram_tensor("tri", [128, 128], BF16, kind="ExternalInput").ap()
    y = nc.dram_tensor("y", [S, D], BF16, kind="ExternalOutput").ap()
    with tile.TileContext(nc) as tc:
        _mhsa_kernel(tc, y, xT, wqT, wkT, wvT, woT, tri)
    nc.compile()
    return nc


def get_compiled():
    global _compiled
    if _compiled is None:
        _compiled = _build()
    return _compiled


def _make_tri():
    # tri[k, q] keeps key k <= query q within a 128x128 diagonal block
    k = np.arange(128)
    return (k[None, :] >= k[:, None]).astype(np.float32).astype(bf16)


def _pack_dtiles(m):
    # [T*128, F] -> [128, T*F]: d-tile-major per partition row
    t = m.shape[0] // 128
    return np.ascontiguousarray(
        m.reshape(t, 128, m.shape[1]).transpose(1, 0, 2).reshape(128, -1))


def _pack_x(xc):
    # [1024, 2048] -> [128, 16384] chunk-major: s[0:512) | s[512:1024) |
    # s[1024:2048), each chunk d-tile-major (matches kernel x_sl layout)
    return np.concatenate([
        _pack_dtiles(np.ascontiguousarray(xc[:, 0:512])),
        _pack_dtiles(np.ascontiguousarray(xc[:, 512:1024])),
        _pack_dtiles(np.ascontiguousarray(xc[:, 1024:2048])),
    ], axis=1)


def make_in_maps(inputs):
    x = np.asarray(inputs["in_features"], dtype=np.float32)
    w_q = np.asarray(inputs["w_q"], dtype=np.float32)
    w_k = np.asarray(inputs["w_k"], dtype=np.float32)
    w_v = np.asarray(inputs["w_v"], dtype=np.float32)
    w_o = np.asarray(inputs["w_o"], dtype=np.float32)
    tri = _make_tri()
    in_maps = []
    for c in range(NCORES):
        b, hg = divmod(c, 2)
        es = slice(hg * E, (hg + 1) * E)
        in_maps.append({
            "xT": _pack_x(x[b].T).astype(bf16),
            "wqT": _pack_dtiles(w_q[es, :].T).astype(bf16),
            "wkT": _pack_dtiles(w_k[es, :].T).astype(bf16),
            "wvT": _pack_dtiles(w_v[es, :].T).astype(bf16),
            "woT": _pack_dtiles(w_o[:, es].T).astype(bf16),
            "tri": tri,
        })
    return in_maps


def kernel(**inputs):
    global last_results
    nc = get_compiled()
    in_maps = make_in_maps(inputs)
    res = run_bass_kernel_spmd(nc, in_maps, list(range(NCORES)))
    last_results = res
    y = np.zeros((B, S, D), dtype=np.float32)
    for c in range(NCORES):
        y[c // 2] += np.asarray(res.results[c]["y"], dtype=np.float32)
    return y


# revision 24
# speedup vs baseline: 1.0640x; 1.0217x over previous
"""Multi-head self-attention (causal) Trainium2 Bass/Tile kernel, 8-way SPMD.

Sharding: data-parallel over batch (4) x tensor-parallel over heads (2 groups
of 8 heads).  Core c handles batch c//2, head-group c%2.  Each core computes
q/k/v projections for its 512 local features, causal attention for its 8
heads, and a partial o-projection (contraction over its 512 features of the
attention output) giving a full-shape [S, D] partial (bf16) that the host
sums per batch pair.

All matmul operands are bf16 (fp32 PSUM accumulation); softmax runs without
max-subtraction (scores ~ N(0,1) after the 1/8 scale, no overflow risk), with
exp on the scalar engine and the row-sum folded into the AV matmul via a ones
column appended to V.  Host pre-transposes inputs so no on-chip transposes
are needed:
  qT[e,s]  = wqT.T @ xT        (lhsT=wqT[d,e], rhs=xT[d,s])
  scoresT[sk,sq] = kT.T @ qT   (lhsT=kT[dk,sk], rhs=qT[dk,sq], K=64)
  avT[dk+1,sq]   = vaug.T @ expT  (lhsT=vaug[sk,65], rhs=expT[sk,sq])
  y[s,e]   = outT.T @ woT      (lhsT=outT[d,s], rhs=woT[d,e])

The attention inner loop is ACT(exp)-cadence-limited (~1.1us per key tile vs
~0.8us of PE fill), so all projection / o-projection matmuls are DRIPPED one
instruction at a time between attention matmuls by a tiny work-queue
scheduler; slot boundaries force-drain whatever an upcoming slot depends on.

Causal structure: off-diagonal key tiles are computed full-width; the 4
diagonal-strip tiles of each query group only compute queries >= the key
tile's start (narrowed scores / exp / AV), with a single shared [128,128]
upper-triangular mask applied to the one partially-masked 128-query chunk.
PSUM per-element has_written semantics make the narrowed AV accumulation
correct in any order (first matmul start=True pending-zeroes the bank).
"""

from contextlib import ExitStack

import numpy as np
import ml_dtypes

import concourse.bass as bass
import concourse.tile as tile
from concourse import bacc, mybir
from concourse._compat import with_exitstack
from concourse.bass_utils import run_bass_kernel_spmd

B, S, D, H = 4, 2048, 1024, 16
DK = D // H          # 64
E = 512              # local features per core (8 heads)
HL = 8               # local heads
NCORES = 8
NDT = D // 128       # 8 d-tiles
NET = E // 128       # 4 e-tiles
NST = S // 128       # 16 s-tiles
NQG = S // 512       # 4 query groups

LAG = 2              # exp -> AV pipeline lag (in key tiles)
N_WARM = 8           # HAM warm-up matmuls
DRIP_RATE = 1.2      # drip steps per off-diagonal key tile

F32 = mybir.dt.float32
BF16 = mybir.dt.bfloat16
bf16 = ml_dtypes.bfloat16

_compiled = None
last_results = None  # test harness introspection


class Drip:
    """FIFO work queue of single-matmul steps, dripped between attention
    matmuls at a budgeted rate; need() force-drains through a keyed item."""

    def __init__(self):
        self.q = []          # list of [key, steps]
        self.done = set()
        self.budget = 0.0

    def add(self, key, steps):
        self.q.append([key, list(steps)])

    def _run_one(self):
        while self.q and not self.q[0][1]:
            self.done.add(self.q.pop(0)[0])
        if not self.q:
            return False
        self.q[0][1].pop(0)()
        if not self.q[0][1]:
            self.done.add(self.q.pop(0)[0])
        return True

    def need(self, key):
        if key in self.done:
            return
        keys = [k for k, _ in self.q]
        assert key in keys, f"drip: {key} not queued"
        while key not in self.done:
            assert self._run_one()

    def drip(self, rate):
        self.budget += rate
        while self.budget >= 1.0:
            if not self._run_one():
                self.budget = 0.0
                return
            self.budget -= 1.0

    def drain_all(self):
        while self._run_one():
            pass


@with_exitstack
def _mhsa_kernel(ctx: ExitStack, tc: tile.TileContext, y, xT, wqT, wkT, wvT,
                 woT, tri):
    nc = tc.nc

    consts = ctx.enter_context(tc.tile_pool(name="consts", bufs=1))
    ex_pool = ctx.enter_context(tc.tile_pool(name="ex", bufs=10))
    rec_pool = ctx.enter_context(tc.tile_pool(name="rec", bufs=2))
    y_pool = ctx.enter_context(tc.tile_pool(name="ysb", bufs=3))
    sc_pool = ctx.enter_context(tc.tile_pool(name="sc", bufs=2, space="PSUM"))
    pj_pool = ctx.enter_context(tc.tile_pool(name="pj", bufs=2, space="PSUM"))
    av_pool = ctx.enter_context(tc.tile_pool(name="av", bufs=2, space="PSUM"))

    def ctile(shape, dt_, tg):
        return consts.tile(shape, dt_, tag=tg, name=tg)

    # ---- persistent SBUF tiles -------------------------------------------
    xT_t = ctile([128, NDT * S], BF16, "xTt")          # [p, dt*2048 + s]
    wqT_t = ctile([128, NDT * E], BF16, "wqTt")        # [p, dt*512 + e]
    wkT_t = ctile([128, NDT * E], BF16, "wkTt")
    wvT_t = ctile([128, NDT * E], BF16, "wvTt")
    woT_t = ctile([128, NET * D], BF16, "woTt")        # [p, et*1024 + d]
    qT_t = [ctile([128, S], BF16, f"qT{i}") for i in range(NET)]
    kT_t = [ctile([128, S], BF16, f"kT{i}") for i in range(NET)]
    vaug_t = [ctile([128, HL * (DK + 1)], BF16, f"vaug{i}") for i in range(NST)]
    outT_t = [ctile([128, S], BF16, f"outT{i}") for i in range(NET)]
    tri_t = ctile([128, 128], BF16, "tri")
    warm = ctile([128, 512], BF16, "warm")

    # x is host-packed chunk-major so every load below is fully contiguous:
    # chunk c0 = s[0:512) at offset 0 (t-major, 512 per t), c1 = s[512:1024)
    # at 4096, c2 = s[1024:2048) at 8192 (1024 per t).  No kernel slice
    # crosses a chunk boundary (all uses are 128-aligned within a 512 chunk).
    def x_sl(dt_, a, b):
        if b <= 512:
            base, tl, off = 0, 512, a
        elif a >= 512 and b <= 1024:
            base, tl, off = 4096, 512, a - 512
        else:
            assert a >= 1024 and b <= 2048, (a, b)
            base, tl, off = 8192, 1024, a - 1024
        p = base + dt_ * tl + off
        return xT_t[:, p:p + (b - a)]

    def wq_sl(dt_, et):
        return wqT_t[:, dt_ * E + et * 128:dt_ * E + (et + 1) * 128]

    def wk_sl(dt_, et):
        return wkT_t[:, dt_ * E + et * 128:dt_ * E + (et + 1) * 128]

    def wv_sl(dt_):
        return wvT_t[:, dt_ * E:(dt_ + 1) * E]

    def wo_sl(et, hf):
        return woT_t[:, et * D + hf * 512:et * D + (hf + 1) * 512]

    # ---- input loads: all host-packed + contiguous.  DMA engines stripe
    # every transfer across 16 engines sharing ~350GB/s, so what matters is
    # PHASING: only first-needed bytes in flight early.  Phase 1: c0 (gpsimd)
    # + wv..c1 FIFO (sync) + tri (scalar).  x chunk c2 and woT (3MB, needed
    # from ~40us) are issued later from boundary(0,3) to keep them from
    # stealing phase-1 bandwidth.
    # warm tile memset on the (idle) vector queue so the HAM warm-up
    # matmuls are not stuck behind DMA issues on gpsimd.
    nc.vector.memset(warm, 0.0)

    # gpsimd FIFO orders the x/weight stream by first-need: c0 halves (v
    # chains), then wq/wk (first q/k chains), then x chunk c1.  wv rides
    # alone on sync so phase 1 is exactly {c0, wv} at full bandwidth.
    nc.gpsimd.dma_start(out=xT_t[:, 0:2048], in_=xT[:, 0:2048])
    nc.gpsimd.dma_start(out=xT_t[:, 2048:4096], in_=xT[:, 2048:4096])
    nc.sync.dma_start(out=wvT_t, in_=wvT)
    nc.gpsimd.dma_start(out=wqT_t, in_=wqT)
    nc.gpsimd.dma_start(out=wkT_t, in_=wkT)
    nc.gpsimd.dma_start(out=xT_t[:, 4096:8192], in_=xT[:, 4096:8192])
    nc.scalar.dma_start(out=tri_t, in_=tri)

    def load_phase2():
        nc.gpsimd.dma_start(out=xT_t[:, 8192:16384], in_=xT[:, 8192:16384])
        nc.gpsimd.dma_start(out=woT_t, in_=woT)

    # ---- PE warm-up: HAM releases after ~3.4us of sustained matmuls ------
    for _ in range(N_WARM):
        wps = pj_pool.tile([128, 512], F32, tag="pj", name="wps")
        nc.tensor.matmul(wps, lhsT=warm[:, 0:128], rhs=warm,
                         start=True, stop=True)

    # ---- projection chains (single-matmul drip steps) --------------------
    def qk_steps(wsl, dst, et, scg, hf):
        steps = []
        state = {}
        for dt_ in range(NDT):
            def step(dt_=dt_):
                if dt_ == 0:
                    state[0] = pj_pool.tile([128, 512], F32, tag="pj",
                                            name="pj")
                ps = state[0]
                s0 = scg * 1024 + hf * 512
                nc.tensor.matmul(
                    ps, lhsT=wsl(dt_, et), rhs=x_sl(dt_, s0, s0 + 512),
                    start=(dt_ == 0), stop=(dt_ == NDT - 1))
                if dt_ == NDT - 1:
                    nc.vector.tensor_copy(dst[et][:, s0:s0 + 512], ps)
            steps.append(step)
        return steps

    def v_steps(st):
        steps = []
        state = {}
        for dt_ in range(NDT):
            def step(dt_=dt_):
                if dt_ == 0:
                    state[0] = pj_pool.tile([128, 512], F32, tag="pj",
                                            name="pj")
                ps = state[0]
                nc.tensor.matmul(
                    ps, lhsT=x_sl(dt_, st * 128, (st + 1) * 128), rhs=wv_sl(dt_),
                    start=(dt_ == 0), stop=(dt_ == NDT - 1))
                if dt_ == NDT - 1:
                    nc.vector.memset(vaug_t[st], 1.0)
                    nc.vector.tensor_copy(
                        vaug_t[st].rearrange("p (h c) -> p h c", c=65)[:, :, 0:64],
                        ps.rearrange("p (h c) -> p h c", c=64))
            steps.append(step)
        return steps

    def o_steps(st):
        steps = []
        state = {}
        for hf in range(2):
            for et in range(NET):
                def step(hf=hf, et=et, st=st):
                    if et == 0:
                        state[hf] = pj_pool.tile([128, 512], F32, tag="pj",
                                                 name="pj")
                    ps = state[hf]
                    nc.tensor.matmul(
                        ps, lhsT=outT_t[et][:, st * 128:(st + 1) * 128],
                        rhs=wo_sl(et, hf),
                        start=(et == 0), stop=(et == NET - 1))
                    if et == NET - 1:
                        ysb = y_pool.tile([128, 512], BF16, tag="ysb",
                                          name="ysb")
                        nc.vector.tensor_copy(ysb, ps)
                        nc.sync.dma_start(
                            out=y[st * 128:(st + 1) * 128,
                                  hf * 512:(hf + 1) * 512],
                            in_=ysb)
                steps.append(step)
        return steps

    # Softmax denominators bounce through DRAM: DVE can only write at
    # 32-aligned base partitions, and SBUF APs cannot have a step-0
    # partition dim (needed for the broadcast) — DRAM APs can do both.
    sums_dram = nc.dram_tensor("sums_bounce", [NQG, HL, 512], F32).ap()
    rec_dram = nc.dram_tensor("rec_bounce", [NQG, HL, 512], BF16).ap()

    # ones2: selector for the final pair's reciprocal broadcast matmul —
    # bc[j, :] = recb2[0, :] for j<64 (head A) and recb2[32, :] for j>=64
    # (head B).  Rows 0/32 because the DVE can only write at 32-aligned
    # partitions; K padded to 64 (a K=33 matmul wedges the exec unit).
    ones2 = ctile([64, 128], BF16, "ones2")
    nc.vector.memset(ones2, 0.0)
    nc.vector.memset(ones2[0:1, 0:64], 1.0)
    nc.vector.memset(ones2[32:33, 64:128], 1.0)

    sched = Drip()

    # ---- attention: one global pipeline over all (qg, hp, kt) -------------
    # Heads hA=2*hp (partitions 0:64) and hB=2*hp+1 (64:128) share each
    # score tile: [:, 0:512]=A, [:, 512:1024]=B for one key tile kt.  The
    # K=64 score matmuls for A and B land on disjoint PE row groups (base
    # partition 0 vs 64) and run concurrently.  outT stays UNNORMALIZED;
    # denominators are collected and normalization is batched per qg.
    # Diagonal-strip tiles (kt-4*qg = j >= 0) are narrowed to queries
    # >= 128*j within the group.  The exp->AV lag spans slot boundaries so
    # the PE never waits on a fresh exp at a slot transition; a slot's
    # stash (and its boundary actions) runs when its last AV pops, ~LAG
    # key tiles into the next slot.
    pend = []

    def emit_av(it):
        kt, ex, qo, avA, avB, qg, hp = it
        nk = 4 * qg + 4
        for av, h in ((avA, 2 * hp), (avB, 2 * hp + 1)):
            nc.tensor.matmul(
                av[:, qo:512],
                lhsT=vaug_t[kt][:, h * 65:h * 65 + 65],
                rhs=ex[:, (h & 1) * 512 + qo:((h & 1) + 1) * 512],
                start=(kt == 0), stop=(kt == nk - 1),
            )
        if kt == nk - 1:
            if (qg, hp) == (NQG - 1, HL // 2 - 1):
                return  # final slot: stash emitted manually in the tail
            _stash(hp, qg, hp, avA, avB)
            boundary(qg, hp)

    def attn(hp, qg):
        ti = hp
        nk = 4 * qg + 4
        avA = av_pool.tile([65, 512], F32, tag="av", name="avA")
        avB = av_pool.tile([65, 512], F32, tag="av", name="avB")
        for kt in range(nk):
            j = kt - 4 * qg          # >=0 on the diagonal strip
            qo = 128 * j if j > 0 else 0
            ps = sc_pool.tile([128, 1024], F32, tag="sc", name="ps")
            for po in (0, 64):
                nc.tensor.matmul(
                    ps[:, (po // 64) * 512 + qo:(po // 64 + 1) * 512],
                    lhsT=kT_t[ti][po:po + 64, kt * 128:(kt + 1) * 128],
                    rhs=qT_t[ti][po:po + 64, qg * 512 + qo:(qg + 1) * 512],
                    start=True, stop=True,
                )
            ex = ex_pool.tile([128, 1024], BF16, tag="ex", name="ex")
            if qo == 0:
                nc.scalar.activation(out=ex, in_=ps,
                                     func=mybir.ActivationFunctionType.Exp,
                                     scale=0.125)
            else:
                for half in range(2):
                    a = half * 512 + qo
                    b = (half + 1) * 512
                    nc.scalar.activation(
                        out=ex[:, a:b], in_=ps[:, a:b],
                        func=mybir.ActivationFunctionType.Exp, scale=0.125)
            if j >= 0:  # triangular mask on the one partial 128-q chunk
                for half in range(2):
                    a = half * 512 + qo
                    nc.vector.tensor_mul(ex[:, a:a + 128], ex[:, a:a + 128],
                                         tri_t)
            pend.append((kt, ex, qo, avA, avB, qg, hp))
            rate = DRIP_RATE if j < 1 else max(0.2, DRIP_RATE - 0.35 * j)
            sched.drip(rate)
            while len(pend) > LAG:
                emit_av(pend.pop(0))

    def _stash(hp, qg, ti, avA, avB):
        # stash unnormalized outputs + denominators; release av quickly
        hA, hB = 2 * hp, 2 * hp + 1
        if qg == NQG - 1 and hp == HL // 2 - 1:
            # final pair: no attention left to hide the DRAM-bounce latency
            # behind, so normalize inline via reciprocal + PE broadcast
            stg2 = rec_pool.tile([64, 512], F32, tag="stg2", name="stg2")
            nc.vector.memset(stg2, 1.0)
            for av, po, row in ((avA, 0, 0), (avB, 64, 32)):
                nc.vector.tensor_copy(
                    outT_t[ti][po:po + 64, qg * 512:(qg + 1) * 512],
                    av[0:64, :])
                nc.vector.tensor_copy(stg2[row:row + 1, :], av[64:65, :])
            rec2 = rec_pool.tile([64, 512], F32, tag="rec2", name="rec2")
            nc.vector.reciprocal_approx_fast(out=rec2, in_=stg2)
            recb2 = rec_pool.tile([64, 512], BF16, tag="recb2", name="recb2")
            nc.vector.tensor_copy(recb2, rec2)
            bc = av_pool.tile([128, 512], F32, tag="av", name="bc")
            nc.tensor.matmul(bc, lhsT=ones2, rhs=recb2, start=True, stop=True)
            for po in (0, 64):
                sl = outT_t[ti][po:po + 64, qg * 512:(qg + 1) * 512]
                # bc lives in PSUM, which gpsimd cannot read — keep on DVE
                nc.vector.tensor_mul(sl, sl, bc[po:po + 64, :])
        else:
            for av, h, po in ((avA, hA, 0), (avB, hB, 64)):
                nc.vector.tensor_copy(
                    outT_t[ti][po:po + 64, qg * 512:(qg + 1) * 512],
                    av[0:64, :])
                stg = rec_pool.tile([1, 512], F32, tag="stg", name="stg",
                                    bufs=4)
                nc.vector.tensor_copy(stg, av[64:65, :])
                nc.sync.dma_start(out=sums_dram[qg, h], in_=stg)

    # ---- batched normalization (DRAM-bounce broadcast) -------------------
    def _norm_heads(qg, heads):
        h0, nh = heads[0], len(heads)
        sums = rec_pool.tile([nh, 512], F32, tag=f"sums{nh}", name="sums")
        nc.sync.dma_start(out=sums, in_=sums_dram[qg, h0:h0 + nh])
        rec = rec_pool.tile([nh, 512], F32, tag=f"rec{nh}", name="rec")
        nc.vector.reciprocal_approx_fast(out=rec, in_=sums)
        recb = rec_pool.tile([nh, 512], BF16, tag=f"recb{nh}", name="recb")
        nc.vector.tensor_copy(recb, rec)
        nc.sync.dma_start(out=rec_dram[qg, h0:h0 + nh], in_=recb)
        for h in heads:
            ti, po = h // 2, 64 * (h % 2)
            # walrus requires SBUF tensor_tensor inputs to share the start
            # partition, so land the broadcast at the same partition range
            bcs = rec_pool.tile([128, 512], BF16, tag="bcs", name="bcs")
            nc.sync.dma_start(
                out=bcs[po:po + 64, :],
                in_=rec_dram[qg, h:h + 1, :].to_broadcast([64, 512]))
            sl = outT_t[ti][po:po + 64, qg * 512:(qg + 1) * 512]
            nc.gpsimd.tensor_mul(sl, sl, bcs[po:po + 64, :])

    def normalize(qg):
        _norm_heads(qg, list(range(HL)))

    def normalize_pair(qg, hp):
        _norm_heads(qg, [2 * hp, 2 * hp + 1])

    # post-slot boundary actions, keyed by (qg, hp) whose stash just ran
    def boundary(qg, hp):
        if qg == 0 and hp == 3:
            load_phase2()
        if qg == 1 and hp == 0:
            normalize(0)
            for st in (0, 1, 2, 3):
                sched.add(("o", st), o_steps(st))
        elif qg == 2 and hp == 0:
            normalize(1)
            for st in (4, 5, 6, 7):
                sched.add(("o", st), o_steps(st))
        elif qg == 3 and hp == 0:
            normalize(2)
            for st in (8, 9, 10, 11):
                sched.add(("o", st), o_steps(st))
        if qg == 3 and hp < 3:
            normalize_pair(3, hp)

    # ---- program order ----------------------------------------------------
    # Pre-loop: everything slot (0,0) depends on.  The drip queue holds the
    # remaining projections in the order later slots need them; oproj steps
    # are appended once their query group is normalized.
    def addq(et, scg, hf):
        sched.add(("q", et, scg, hf), qk_steps(wq_sl, qT_t, et, scg, hf))

    def addk(et, scg, hf):
        sched.add(("k", et, scg, hf), qk_steps(wk_sl, kT_t, et, scg, hf))

    for st in range(4):
        sched.add(("v", st), v_steps(st))
    addq(0, 0, 0)
    addk(0, 0, 0)
    sched.need(("v", 3))
    sched.need(("q", 0, 0, 0))
    sched.need(("k", 0, 0, 0))

    for et in (1, 2, 3):
        addq(et, 0, 0)
        addk(et, 0, 0)
    for st in (4, 5, 6, 7):
        sched.add(("v", st), v_steps(st))
    for et in (0, 1, 2, 3):
        addk(et, 0, 1)
        addq(et, 0, 1)
    for st in (8, 9, 10, 11):
        sched.add(("v", st), v_steps(st))
    for et in (0, 1, 2, 3):
        addq(et, 1, 0)
        addk(et, 1, 0)
    for st in (12, 13, 14, 15):
        sched.add(("v", st), v_steps(st))
    for et in (0, 1, 2, 3):
        addk(et, 1, 1)
        addq(et, 1, 1)

    for qg in range(NQG):
        for hp in range(HL // 2):
            sched.need(("q", hp, qg // 2, qg % 2))
            for scg in range(2):
                for hf in range(2):
                    if 2 * scg + hf <= qg:
                        sched.need(("k", hp, scg, hf))
            sched.need(("v", min(4 * qg + 3, NST - 1)))
            attn(hp, qg)

    # Final drain: the last two AVs of (3,3) wait on the freshest exps and
    # the stash's inline normalization is a serial DVE chain; drip queued
    # oproj steps into both wait windows so the PE stays fed, and emit the
    # final stash manually between them.
    assert len(pend) == LAG
    emit_av(pend.pop(0))
    for _ in range(5):
        sched._run_one()
    it15 = pend.pop(0)
    emit_av(it15)
    for _ in range(8):
        sched._run_one()
    _stash(HL // 2 - 1, NQG - 1, HL // 2 - 1, it15[3], it15[4])
    sched.drain_all()
    for st in (12, 13, 14, 15):
        sched.add(("o", st), o_steps(st))
    sched.drain_all()


def _build():
    nc = bacc.Bacc("TRN2", target_bir_lowering=False, debug=False,
                   num_devices=NCORES)
    # All inputs host-packed to the exact SBUF tile layouts (partition-major,
    # fully contiguous per partition) so every load DMA runs at full rate.
    xT = nc.dram_tensor("xT", [128, NDT * S], BF16, kind="ExternalInput").ap()
    wqT = nc.dram_tensor("wqT", [128, NDT * E], BF16, kind="ExternalInput").ap()
    wkT = nc.dram_tensor("wkT", [128, NDT * E], BF16, kind="ExternalInput").ap()
    wvT = nc.dram_tensor("wvT", [128, NDT * E], BF16, kind="ExternalInput").ap()
    woT = nc.dram_tensor("woT", [128, NET * D], BF16, kind="ExternalInput").ap()
    tri = nc.dram_tensor("tri", [128, 128], BF16, kind="ExternalInput").ap()
    y = nc.dram_tensor("y", [S, D], BF16, kind="ExternalOutput").ap()
    with tile.TileContext(nc) as tc:
        _mhsa_kernel(tc, y, xT, wqT, wkT, wvT, woT, tri)
    nc.compile()
    return nc


def get_compiled():
    global _compiled
    if _compiled is None:
        _compiled = _build()
    return _compiled


def _make_tri():
    # tri[k, q] keeps key k <= query q within a 128x128 diagonal block
    k = np.arange(128)
    return (k[None, :] >= k[:, None]).astype(np.float32).astype(bf16)


def _pack_dtiles(m):
    # [T*128, F] -> [128, T*F]: d-tile-major per partition row
    t = m.shape[0] // 128
    return np.ascontiguousarray(
        m.reshape(t, 128, m.shape[1]).transpose(1, 0, 2).reshape(128, -1))


def _pack_x(xc):
    # [1024, 2048] -> [128, 16384] chunk-major: s[0:512) | s[512:1024) |
    # s[1024:2048), each chunk d-tile-major (matches kernel x_sl layout)
    return np.concatenate([
        _pack_dtiles(np.ascontiguousarray(xc[:, 0:512])),
        _pack_dtiles(np.ascontiguousarray(xc[:, 512:1024])),
        _pack_dtiles(np.ascontiguousarray(xc[:, 1024:2048])),
    ], axis=1)


def make_in_maps(inputs):
    x = np.asarray(inputs["in_features"], dtype=np.float32)
    w_q = np.asarray(inputs["w_q"], dtype=np.float32)
    w_k = np.asarray(inputs["w_k"], dtype=np.float32)
    w_v = np.asarray(inputs["w_v"], dtype=np.float32)
    w_o = np.asarray(inputs["w_o"], dtype=np.float32)
    tri = _make_tri()
    in_maps = []
    for c in range(NCORES):
        b, hg = divmod(c, 2)
        es = slice(hg * E, (hg + 1) * E)
        in_maps.append({
            "xT": _pack_x(x[b].T).astype(bf16),
            "wqT": _pack_dtiles(w_q[es, :].T).astype(bf16),
            "wkT": _pack_dtiles(w_k[es, :].T).astype(bf16),
            "wvT": _pack_dtiles(w_v[es, :].T).astype(bf16),
            "woT": _pack_dtiles(w_o[:, es].T).astype(bf16),
            "tri": tri,
        })
    return in_maps


def kernel(**inputs):
    global last_results
    nc = get_compiled()
    in_maps = make_in_maps(inputs)
    res = run_bass_kernel_spmd(nc, in_maps, list(range(NCORES)))
    last_results = res
    y = np.zeros((B, S, D), dtype=np.float32)
    for c in range(NCORES):
        y[c // 2] += np.asarray(res.results[c]["y"], dtype=np.float32)
    return y
